# revision 40
# baseline (speedup 1.0000x reference)
"""ARX cell (nn_ARXCell) Trainium2 kernel.

Computes, for each batch row b:
    a2[b]       = dot(iw, itdl[b]) + dot(ow, otdl[b]) + bias
    itdl_new[b] = concat(itdl[b, 64:], input[b])          # shift reg, shift=64
    otdl_new[b] = concat(otdl[b, 1:], a2[b])              # shift reg, shift=1
    outputs[b]  = a2[b]

Pure data parallel over 8 NeuronCores: batch 16384 -> 2048 rows/core.
Per core the 16 MiB itdl shard is streamed through SBUF exactly once:
each [128, 2048] tile is DMA'd in (SP/HWDGE ring, pure reads), DMA'd
back out shifted (ACT/HWDGE ring, pure writes — the two rings feed
disjoint SDMA banks, and keeping reads and writes on separate banks
avoids HBM bus-turnaround penalties), and consumed by the DVE dot
product (mul + reduce). Small tensors are staged once via GPSIMD/SWDGE.
Raw Bass with explicit semaphores; NBUF-deep buffering paces the load
stream to DVE consumption.

Rows are mapped partition-major within a core: row b = p*T + t lives in
partition p, tile t, which makes every small DRAM access pattern
contiguous-ish (outputs: 64B runs; otdl_new: 4KB runs after SBUF
assembly).
"""

from contextlib import ExitStack

import numpy as np

import concourse.bass as bass
import concourse.mybir as mybir
from concourse.bass_utils import run_bass_kernel_spmd

# Problem shapes (hardcoded per harness contract).
BATCH = 16384
INPUT_SIZE = 64  # new samples appended to itdl per step
DI = 2048        # input delay line length (32 * 64)
DO = 64          # output delay line length (64 * 1)
N_CORES = 8
B_SHARD = BATCH // N_CORES  # 2048 rows per core
P = 128                     # SBUF partitions
T = B_SHARD // P            # 16 row-tiles per core
SHIFT = INPUT_SIZE          # 64
KEEP = DI - SHIFT           # 1984
NBUF = 8                    # itdl tile ring depth

F32 = mybir.dt.float32

_NC_CACHE = None


def build_nc() -> bass.Bass:
    nc = bass.Bass()

    inp = nc.declare_dram_parameter("input", [B_SHARD, INPUT_SIZE], F32, isOutput=False)
    itdl = nc.declare_dram_parameter("itdl", [B_SHARD, DI], F32, isOutput=False)
    otdl = nc.declare_dram_parameter("otdl", [B_SHARD, DO], F32, isOutput=False)
    iw = nc.declare_dram_parameter("iw", [1, DI], F32, isOutput=False)
    ow = nc.declare_dram_parameter("ow", [1, DO], F32, isOutput=False)
    b = nc.declare_dram_parameter("b", [1, 1], F32, isOutput=False)
    outputs = nc.declare_dram_parameter("outputs", [B_SHARD, 1], F32, isOutput=True)
    itdl_new = nc.declare_dram_parameter("itdl_new", [B_SHARD, DI], F32, isOutput=True)
    otdl_new = nc.declare_dram_parameter("otdl_new", [B_SHARD, DO], F32, isOutput=True)

    # DRAM views: row b = p*T + t lives at [p, t, :] (partition-major).
    inp_3d = inp[:, :].rearrange("(p t) c -> p t c", p=P)
    otdl_3d = otdl[:, :].rearrange("(p t) c -> p t c", p=P)
    it_3d = itdl[:, :].rearrange("(p t) c -> p t c", p=P)
    itn_3d = itdl_new[:, :].rearrange("(p t) c -> p t c", p=P)
    otn_3d = otdl_new[:, :].rearrange("(p t) c -> p t c", p=P)
    outputs_2d = outputs[:, :].rearrange("(p t) one -> p (t one)", p=P)

    with ExitStack() as ctx:
        iw_b = ctx.enter_context(nc.sbuf_tensor([P, DI], F32))
        ow_b = ctx.enter_context(nc.sbuf_tensor([P, DO], F32))
        b_b = ctx.enter_context(nc.sbuf_tensor([P, 1], F32))
        in_sb = ctx.enter_context(nc.sbuf_tensor([P, T, INPUT_SIZE], F32))
        ot_sb = ctx.enter_context(nc.sbuf_tensor([P, T, DO], F32))
        ot_new = ctx.enter_context(nc.sbuf_tensor([P, T, DO], F32))
        oprod = ctx.enter_context(nc.sbuf_tensor([P, T, DO], F32))
        obias = ctx.enter_context(nc.sbuf_tensor([P, T], F32))
        araw = ctx.enter_context(nc.sbuf_tensor([P, T], F32))
        a2 = ctx.enter_context(nc.sbuf_tensor([P, T], F32))
        itbuf = ctx.enter_context(nc.sbuf_tensor([P, NBUF, DI], F32))
        prod = ctx.enter_context(nc.sbuf_tensor([P, 2, DI], F32))
        # Per-slot DMA semaphores: at most ONE DMA in flight per semaphore,
        # so cumulative multiples of 16 are exact completion milestones
        # (a shared counter would interleave partial increments from
        # concurrent DMAs).
        sem_iw = ctx.enter_context(nc.semaphore("sem_iw"))  # iw_b, wait at 16
        sem_gp = ctx.enter_context(nc.semaphore("sem_gp"))  # staging, wait 64
        sem_ld = [
            ctx.enter_context(nc.semaphore(f"sem_ld{i}")) for i in range(NBUF)
        ]
        sem_st = [
            ctx.enter_context(nc.semaphore(f"sem_st{i}")) for i in range(NBUF)
        ]
        sem_dve = ctx.enter_context(nc.semaphore("sem_dve"))  # DVE op counter
        sem_ost = ctx.enter_context(nc.semaphore("sem_ost"))  # small out stores
        block = ctx.enter_context(nc.Block())

        # DVE producer numbering (value of sem_dve after each op):
        #   2t+1: mul prod_t    2t+2: reduce araw_t     (t = 0..T-1)
        #   2T+1: mul oprod     2T+2: reduce obias
        #   2T+3: a2 = araw+obias   2T+4: a2 += b
        #   2T+5: ot_new shift copy 2T+6: ot_new[..,63] = a2
        DVE_MUL = lambda t: 2 * t + 1
        DVE_DONE = 2 * T + 6
        # Load/store of tile t use slot t%NBUF; cycle c = t//NBUF completes
        # that slot's semaphore at (c+1)*16.
        LD_DONE = lambda t: (t // NBUF + 1) * 16

        @block.gpsimd
        def _(gpsimd):
            # Replicated parameters, broadcast to all 128 partitions via DMA.
            gpsimd.dma_start(
                out=iw_b[:], in_=iw[0:1, :].broadcast_to([P, DI])
            ).then_inc(sem_iw, 16)
            gpsimd.dma_start(
                out=ow_b[:], in_=ow[0:1, :].broadcast_to([P, DO])
            ).then_inc(sem_gp, 16)
            gpsimd.dma_start(
                out=b_b[:], in_=b[0:1, :].broadcast_to([P, 1])
            ).then_inc(sem_gp, 16)
            gpsimd.dma_start(out=in_sb[:], in_=inp_3d).then_inc(sem_gp, 16)
            gpsimd.dma_start(out=ot_sb[:], in_=otdl_3d).then_inc(sem_gp, 16)

        @block.sync
        def _(sync):
            # itdl tile loads, NBUF-deep ring (pure read stream on bank A).
            for t in range(T):
                if t >= NBUF:
                    # Slot t-NBUF must be released by DVE (mul) and ACT
                    # (main store) before reuse.
                    sync.wait_ge(sem_dve, DVE_MUL(t - NBUF))
                    sync.wait_ge(sem_st[t % NBUF], LD_DONE(t - NBUF))
                sync.dma_start(
                    out=itbuf[:, t % NBUF, :], in_=it_3d[:, t, :]
                ).then_inc(sem_ld[t % NBUF], 16)

        @block.vector
        def _(vector):
            vector.wait_ge(sem_iw, 16)
            for t in range(T):
                vector.wait_ge(sem_ld[t % NBUF], LD_DONE(t))
                vector.tensor_mul(
                    out=prod[:, t % 2, :], in0=itbuf[:, t % NBUF, :], in1=iw_b[:]
                ).then_inc(sem_dve, 1)
                vector.wait_ge(sem_dve, DVE_MUL(t))
                vector.tensor_reduce(
                    out=araw[:, t : t + 1],
                    in_=prod[:, t % 2, :],
                    axis=mybir.AxisListType.X,
                    op=mybir.AluOpType.add,
                ).then_inc(sem_dve, 1)
            # otdl dot for all tiles at once: obias[p, t] = sum(otdl*ow)
            vector.wait_ge(sem_gp, 64)
            vector.tensor_mul(
                out=oprod[:],
                in0=ot_sb[:],
                in1=ow_b[:, None, :].broadcast_to([P, T, DO]),
            ).then_inc(sem_dve, 1)
            vector.wait_ge(sem_dve, 2 * T + 1)
            vector.tensor_reduce(
                out=obias[:],
                in_=oprod[:],
                axis=mybir.AxisListType.X,
                op=mybir.AluOpType.add,
            ).then_inc(sem_dve, 1)
            # a2 = araw + obias + b
            vector.wait_ge(sem_dve, 2 * T + 2)
            vector.tensor_add(out=a2[:], in0=araw[:], in1=obias[:]).then_inc(
                sem_dve, 1
            )
            vector.wait_ge(sem_dve, 2 * T + 3)
            vector.tensor_add(
                out=a2[:], in0=a2[:], in1=b_b[:, 0:1].broadcast_to([P, T])
            ).then_inc(sem_dve, 1)
            # Assemble otdl_new in SBUF: shift by one, append a2.
            vector.tensor_copy(
                out=ot_new[:, :, 0 : DO - 1], in_=ot_sb[:, :, 1:DO]
            ).then_inc(sem_dve, 1)
            vector.wait_ge(sem_dve, 2 * T + 4)
            vector.tensor_copy(
                out=ot_new[:, :, DO - 1 : DO], in_=a2[:, :].unsqueeze(2)
            ).then_inc(sem_dve, 1)

        @block.scalar
        def _(scalar):
            # Shifted main part of itdl_new, tile by tile (pure write
            # stream on bank B).
            for t in range(T):
                scalar.wait_ge(sem_ld[t % NBUF], LD_DONE(t))
                scalar.dma_start(
                    out=itn_3d[:, t, 0:KEEP],
                    in_=itbuf[:, t % NBUF, SHIFT:DI],
                ).then_inc(sem_st[t % NBUF], 16)
            # Tail of itdl_new from the staged input.
            scalar.wait_ge(sem_gp, 64)
            scalar.dma_start(out=itn_3d[:, :, KEEP:DI], in_=in_sb[:]).then_inc(
                sem_ost, 16
            )
            # a2-dependent stores.
            scalar.wait_ge(sem_dve, DVE_DONE)
            scalar.dma_start(out=otn_3d, in_=ot_new[:]).then_inc(sem_ost, 16)
            with nc.allow_non_contiguous_dma(reason="8KiB outputs store"):
                scalar.dma_start(out=outputs_2d, in_=a2[:]).then_inc(sem_ost, 16)
            # Ensure every store landed before the program ends.
            for i in range(NBUF):
                scalar.wait_ge(sem_st[i], (T // NBUF) * 16)
            scalar.wait_ge(sem_ost, 48)

    return nc


def get_nc() -> bass.Bass:
    global _NC_CACHE
    if _NC_CACHE is None:
        _NC_CACHE = build_nc()
    return _NC_CACHE


def shard_inputs(inputs: dict) -> list[dict]:
    inp = np.ascontiguousarray(np.asarray(inputs["input"], dtype=np.float32))
    itdl = np.ascontiguousarray(np.asarray(inputs["itdl"], dtype=np.float32)).reshape(
        BATCH, DI
    )
    otdl = np.ascontiguousarray(np.asarray(inputs["otdl"], dtype=np.float32)).reshape(
        BATCH, DO
    )
    iw = np.ascontiguousarray(np.asarray(inputs["iw"], dtype=np.float32)).reshape(1, DI)
    ow = np.ascontiguousarray(np.asarray(inputs["ow"], dtype=np.float32)).reshape(1, DO)
    b = np.ascontiguousarray(np.asarray(inputs["b"], dtype=np.float32)).reshape(1, 1)

    in_maps = []
    for c in range(N_CORES):
        s = slice(c * B_SHARD, (c + 1) * B_SHARD)
        in_maps.append(
            {
                "input": inp[s],
                "itdl": itdl[s],
                "otdl": otdl[s],
                "iw": iw,
                "ow": ow,
                "b": b,
            }
        )
    return in_maps


def gather_outputs(results: list[dict]) -> tuple[np.ndarray, np.ndarray, np.ndarray]:
    outputs = np.concatenate([r["outputs"] for r in results], axis=0).reshape(
        BATCH, 1, 1
    )
    itdl_new = np.concatenate([r["itdl_new"] for r in results], axis=0).reshape(
        BATCH, DI, 1
    )
    otdl_new = np.concatenate([r["otdl_new"] for r in results], axis=0).reshape(
        BATCH, DO, 1
    )
    return outputs, itdl_new, otdl_new


def kernel(**inputs):
    in_maps = shard_inputs(inputs)
    res = run_bass_kernel_spmd(get_nc(), in_maps, list(range(N_CORES)))
    return gather_outputs(res.results)


# revision 44
# speedup vs baseline: 1.1576x; 1.1576x over previous
"""ARX cell (nn_ARXCell) Trainium2 kernel.

Computes, for each batch row b:
    a2[b]       = dot(iw, itdl[b]) + dot(ow, otdl[b]) + bias
    itdl_new[b] = concat(itdl[b, 64:], input[b])          # shift reg, shift=64
    otdl_new[b] = concat(otdl[b, 1:], a2[b])              # shift reg, shift=1
    outputs[b]  = a2[b]

Pure data parallel over 8 NeuronCores: batch 16384 -> 2048 rows/core.
Per core the 16 MiB itdl shard is streamed through SBUF exactly once:
each [128, 2048] tile is DMA'd in (SP/HWDGE ring, pure reads), DMA'd
back out shifted (ACT/HWDGE ring, pure writes — the two rings feed
disjoint SDMA banks, and keeping reads and writes on separate banks
avoids HBM bus-turnaround penalties), and consumed by the DVE dot
product (mul + reduce). Small tensors are staged once via GPSIMD/SWDGE.
Raw Bass with explicit semaphores; NBUF-deep buffering paces the load
stream to DVE consumption.

Rows are mapped partition-major within a core: row b = p*T + t lives in
partition p, tile t, which makes every small DRAM access pattern
contiguous-ish (outputs: 64B runs; otdl_new: 4KB runs after SBUF
assembly).
"""

from contextlib import ExitStack

import numpy as np

import concourse.bass as bass
import concourse.mybir as mybir
from concourse.bass_utils import run_bass_kernel_spmd

# Problem shapes (hardcoded per harness contract).
BATCH = 16384
INPUT_SIZE = 64  # new samples appended to itdl per step
DI = 2048        # input delay line length (32 * 64)
DO = 64          # output delay line length (64 * 1)
N_CORES = 8
B_SHARD = BATCH // N_CORES  # 2048 rows per core
P = 128                     # SBUF partitions
T = B_SHARD // P            # 16 row-tiles per core
SHIFT = INPUT_SIZE          # 64
KEEP = DI - SHIFT           # 1984
NBUF = 8                    # itdl tile ring depth

F32 = mybir.dt.float32

_NC_CACHE = None


def build_nc() -> bass.Bass:
    nc = bass.Bass()

    inp = nc.declare_dram_parameter("input", [B_SHARD, INPUT_SIZE], F32, isOutput=False)
    itdl = nc.declare_dram_parameter("itdl", [B_SHARD, DI], F32, isOutput=False)
    otdl = nc.declare_dram_parameter("otdl", [B_SHARD, DO], F32, isOutput=False)
    # iw arrives pre-replicated to 128 partitions from the host: a DMA that
    # broadcast-reads one DRAM row 128x serializes on a single HBM bank.
    iw = nc.declare_dram_parameter("iw", [P, DI], F32, isOutput=False)
    ow = nc.declare_dram_parameter("ow", [1, DO], F32, isOutput=False)
    b = nc.declare_dram_parameter("b", [1, 1], F32, isOutput=False)
    outputs = nc.declare_dram_parameter("outputs", [B_SHARD, 1], F32, isOutput=True)
    itdl_new = nc.declare_dram_parameter("itdl_new", [B_SHARD, DI], F32, isOutput=True)
    otdl_new = nc.declare_dram_parameter("otdl_new", [B_SHARD, DO], F32, isOutput=True)

    # DRAM views: row b = p*T + t lives at [p, t, :] (partition-major).
    inp_3d = inp[:, :].rearrange("(p t) c -> p t c", p=P)
    otdl_3d = otdl[:, :].rearrange("(p t) c -> p t c", p=P)
    it_3d = itdl[:, :].rearrange("(p t) c -> p t c", p=P)
    itn_3d = itdl_new[:, :].rearrange("(p t) c -> p t c", p=P)
    otn_3d = otdl_new[:, :].rearrange("(p t) c -> p t c", p=P)
    outputs_2d = outputs[:, :].rearrange("(p t) one -> p (t one)", p=P)

    with ExitStack() as ctx:
        iw_b = ctx.enter_context(nc.sbuf_tensor([P, DI], F32))
        ow_b = ctx.enter_context(nc.sbuf_tensor([P, DO], F32))
        b_b = ctx.enter_context(nc.sbuf_tensor([P, 1], F32))
        in_sb = ctx.enter_context(nc.sbuf_tensor([P, T, INPUT_SIZE], F32))
        ot_sb = ctx.enter_context(nc.sbuf_tensor([P, T, DO], F32))
        ot_new = ctx.enter_context(nc.sbuf_tensor([P, T, DO], F32))
        oprod = ctx.enter_context(nc.sbuf_tensor([P, T, DO], F32))
        obias = ctx.enter_context(nc.sbuf_tensor([P, T], F32))
        araw = ctx.enter_context(nc.sbuf_tensor([P, T], F32))
        a2 = ctx.enter_context(nc.sbuf_tensor([P, T], F32))
        itbuf = ctx.enter_context(nc.sbuf_tensor([P, NBUF, DI], F32))
        prod = ctx.enter_context(nc.sbuf_tensor([P, 2, DI], F32))
        # Per-slot DMA semaphores: at most ONE DMA in flight per semaphore,
        # so cumulative multiples of 16 are exact completion milestones
        # (a shared counter would interleave partial increments from
        # concurrent DMAs).
        sem_iw = ctx.enter_context(nc.semaphore("sem_iw"))  # iw_b, wait at 16
        sem_gp = ctx.enter_context(nc.semaphore("sem_gp"))  # staging, wait 64
        sem_ld = [
            ctx.enter_context(nc.semaphore(f"sem_ld{i}")) for i in range(NBUF)
        ]
        sem_st = [
            ctx.enter_context(nc.semaphore(f"sem_st{i}")) for i in range(NBUF)
        ]
        sem_dve = ctx.enter_context(nc.semaphore("sem_dve"))  # DVE op counter
        sem_ost = ctx.enter_context(nc.semaphore("sem_ost"))  # small out stores
        block = ctx.enter_context(nc.Block())

        # DVE producer numbering (value of sem_dve after each op):
        #   2t+1: mul prod_t    2t+2: reduce araw_t     (t = 0..T-1)
        #   2T+1: mul oprod     2T+2: reduce obias
        #   2T+3: a2 = araw+obias   2T+4: a2 += b
        #   2T+5: ot_new shift copy 2T+6: ot_new[..,63] = a2
        DVE_MUL = lambda t: 2 * t + 1
        DVE_DONE = 2 * T + 6
        # Load/store of tile t use slot t%NBUF; cycle c = t//NBUF completes
        # that slot's semaphore at (c+1)*16.
        LD_DONE = lambda t: (t // NBUF + 1) * 16

        @block.gpsimd
        def _(gpsimd):
            # Small replicated parameters + staging on the SWDGE path.
            gpsimd.dma_start(
                out=ow_b[:], in_=ow[0:1, :].broadcast_to([P, DO])
            ).then_inc(sem_gp, 16)
            gpsimd.dma_start(
                out=b_b[:], in_=b[0:1, :].broadcast_to([P, 1])
            ).then_inc(sem_gp, 16)
            gpsimd.dma_start(out=in_sb[:], in_=inp_3d).then_inc(sem_gp, 16)
            gpsimd.dma_start(out=ot_sb[:], in_=otdl_3d).then_inc(sem_gp, 16)

        @block.sync
        def _(sync):
            # iw_b first: a plain contiguous 1MiB read that gates DVE; on
            # the SP ring it completes ~7us (SWDGE would time-slice it
            # against the bulk HWDGE traffic until ~19us).
            sync.dma_start(out=iw_b[:], in_=iw[:, :]).then_inc(sem_iw, 16)
            # itdl tile loads, NBUF-deep ring (pure read stream on bank A).
            for t in range(T):
                if t >= NBUF:
                    # Slot t-NBUF must be released by DVE (mul) and ACT
                    # (main store) before reuse.
                    sync.wait_ge(sem_dve, DVE_MUL(t - NBUF))
                    sync.wait_ge(sem_st[t % NBUF], LD_DONE(t - NBUF))
                sync.dma_start(
                    out=itbuf[:, t % NBUF, :], in_=it_3d[:, t, :]
                ).then_inc(sem_ld[t % NBUF], 16)

        @block.vector
        def _(vector):
            vector.wait_ge(sem_iw, 16)
            for t in range(T):
                vector.wait_ge(sem_ld[t % NBUF], LD_DONE(t))
                vector.tensor_mul(
                    out=prod[:, t % 2, :], in0=itbuf[:, t % NBUF, :], in1=iw_b[:]
                ).then_inc(sem_dve, 1)
                vector.wait_ge(sem_dve, DVE_MUL(t))
                vector.tensor_reduce(
                    out=araw[:, t : t + 1],
                    in_=prod[:, t % 2, :],
                    axis=mybir.AxisListType.X,
                    op=mybir.AluOpType.add,
                ).then_inc(sem_dve, 1)
            # otdl dot for all tiles at once: obias[p, t] = sum(otdl*ow)
            vector.wait_ge(sem_gp, 64)
            vector.tensor_mul(
                out=oprod[:],
                in0=ot_sb[:],
                in1=ow_b[:, None, :].broadcast_to([P, T, DO]),
            ).then_inc(sem_dve, 1)
            vector.wait_ge(sem_dve, 2 * T + 1)
            vector.tensor_reduce(
                out=obias[:],
                in_=oprod[:],
                axis=mybir.AxisListType.X,
                op=mybir.AluOpType.add,
            ).then_inc(sem_dve, 1)
            # a2 = araw + obias + b
            vector.wait_ge(sem_dve, 2 * T + 2)
            vector.tensor_add(out=a2[:], in0=araw[:], in1=obias[:]).then_inc(
                sem_dve, 1
            )
            vector.wait_ge(sem_dve, 2 * T + 3)
            vector.tensor_add(
                out=a2[:], in0=a2[:], in1=b_b[:, 0:1].broadcast_to([P, T])
            ).then_inc(sem_dve, 1)
            # Assemble otdl_new in SBUF: shift by one, append a2.
            vector.tensor_copy(
                out=ot_new[:, :, 0 : DO - 1], in_=ot_sb[:, :, 1:DO]
            ).then_inc(sem_dve, 1)
            vector.wait_ge(sem_dve, 2 * T + 4)
            vector.tensor_copy(
                out=ot_new[:, :, DO - 1 : DO], in_=a2[:, :].unsqueeze(2)
            ).then_inc(sem_dve, 1)

        @block.scalar
        def _(scalar):
            # Shifted main part of itdl_new, tile by tile (pure write
            # stream on bank B).
            for t in range(T):
                scalar.wait_ge(sem_ld[t % NBUF], LD_DONE(t))
                scalar.dma_start(
                    out=itn_3d[:, t, 0:KEEP],
                    in_=itbuf[:, t % NBUF, SHIFT:DI],
                ).then_inc(sem_st[t % NBUF], 16)
            # Tail of itdl_new from the staged input.
            scalar.wait_ge(sem_gp, 64)
            scalar.dma_start(out=itn_3d[:, :, KEEP:DI], in_=in_sb[:]).then_inc(
                sem_ost, 16
            )
            # a2-dependent stores.
            scalar.wait_ge(sem_dve, DVE_DONE)
            scalar.dma_start(out=otn_3d, in_=ot_new[:]).then_inc(sem_ost, 16)
            with nc.allow_non_contiguous_dma(reason="8KiB outputs store"):
                scalar.dma_start(out=outputs_2d, in_=a2[:]).then_inc(sem_ost, 16)
            # Ensure every store landed before the program ends.
            for i in range(NBUF):
                scalar.wait_ge(sem_st[i], (T // NBUF) * 16)
            scalar.wait_ge(sem_ost, 48)

    return nc


def get_nc() -> bass.Bass:
    global _NC_CACHE
    if _NC_CACHE is None:
        _NC_CACHE = build_nc()
    return _NC_CACHE


def shard_inputs(inputs: dict) -> list[dict]:
    inp = np.ascontiguousarray(np.asarray(inputs["input"], dtype=np.float32))
    itdl = np.ascontiguousarray(np.asarray(inputs["itdl"], dtype=np.float32)).reshape(
        BATCH, DI
    )
    otdl = np.ascontiguousarray(np.asarray(inputs["otdl"], dtype=np.float32)).reshape(
        BATCH, DO
    )
    iw = np.ascontiguousarray(
        np.broadcast_to(
            np.asarray(inputs["iw"], dtype=np.float32).reshape(1, DI), (P, DI)
        )
    )
    ow = np.ascontiguousarray(np.asarray(inputs["ow"], dtype=np.float32)).reshape(1, DO)
    b = np.ascontiguousarray(np.asarray(inputs["b"], dtype=np.float32)).reshape(1, 1)

    in_maps = []
    for c in range(N_CORES):
        s = slice(c * B_SHARD, (c + 1) * B_SHARD)
        in_maps.append(
            {
                "input": inp[s],
                "itdl": itdl[s],
                "otdl": otdl[s],
                "iw": iw,
                "ow": ow,
                "b": b,
            }
        )
    return in_maps


def gather_outputs(results: list[dict]) -> tuple[np.ndarray, np.ndarray, np.ndarray]:
    outputs = np.concatenate([r["outputs"] for r in results], axis=0).reshape(
        BATCH, 1, 1
    )
    itdl_new = np.concatenate([r["itdl_new"] for r in results], axis=0).reshape(
        BATCH, DI, 1
    )
    otdl_new = np.concatenate([r["otdl_new"] for r in results], axis=0).reshape(
        BATCH, DO, 1
    )
    return outputs, itdl_new, otdl_new


def kernel(**inputs):
    in_maps = shard_inputs(inputs)
    res = run_bass_kernel_spmd(get_nc(), in_maps, list(range(N_CORES)))
    return gather_outputs(res.results)
